# revision 20
# baseline (speedup 1.0000x reference)
"""DiffusionGraphConv Trainium2 kernel (8-core SPMD, data-parallel over batch).

Math refactoring (halves the big-matmul FLOPs vs the reference order):
  reference: out[b,n,o] = sum_{f,m} mats_m[n,f,b] * W[f*5+m, o]
  with mats = [x0, s0 x0, 2 s0^2 x0 - x0, s1 x0, 2 s1^2 x0 - x0].
  Projection (width F=128 -> O=64) commutes with the node-space diffusion, so
  with u_m = x0 @ W_m:
    out = (u0 - u2 - u4) + s0 (u1 + 2 s0 u2) + s1 (u3 + 2 s1 u4)
        = v0 + s0 @ c0 + s1 @ c1,  c0 = u1 + s0 @ (2 u2), c1 = u3 + s1 @ (2 u4)

Execution structure (v3 — PE-engine-saturated):
  * u1 / u3 / v0 are never materialized: they are folded into the diffusion
    PSUM groups as 8 extra 64-col matmuls (lhsT = x0 tile, rhs = a pre-scaled
    W slice) appended after the fp8 DoubleRow strip matmuls.  Each hop ends
    in ONE psum->SBUF copy.
  * Dual x0 image: phase 1 and the hop folds read an fp8 x0 (2MB, loaded
    first so the startup window is DMA-short); only the final phase's v0
    folds — the accuracy-dominant direct term — use the bf16 x0, which loads
    after the strips and arrives long before it is needed.  (u2/u4 are
    diffused twice and u1/u3 once, so their fp8 x0 noise averages out.)
  * Both support strip sets stay SBUF-resident (loaded once, 64KB/partition):
    strip HBM traffic is 8MB/core and the final phase has no DMA dependence.
  * PSUM->SBUF drains rotate across DVE / Activation.
  * Output is written bf16 (host accumulates in f32 and adds biases).

Per-core PE work: 512 fp8-DoubleRow matmuls (the 54.6us streaming floor)
+ 512 small matmuls; PE engine busy ~72us is the modeled bottleneck.

Scales (fp8-safety): strips = fp8(256*s); phase1 psum = 2*u{2,4}, drained
with scale 1/16 -> slots fp8(2u/16); hop psum = 16*c{0,1} stored fp8 (fold
rhs = 16*W{1,3} bf16); final psum = 256*16*(s@c) + 4096*v0 = 4096*out,
drained with a 1/4096 scaled copy.

Env quirks handled here: walrus accepts <=1 sync-wait per instruction
(_legalize_waits hoists extras onto EventSemaphore carriers; simulators need
legalize=False); repeat=N re-runs the idempotent pipeline for wall-clock
differencing since this axon terminal has no NTFF profiling.
"""

import sys

if "/opt/trn_rl_repo" not in sys.path:
    sys.path.insert(0, "/opt/trn_rl_repo")

import numpy as np
import ml_dtypes

import concourse.bass as bass
import concourse.mybir as mybir
from concourse.tile import TileContext
from concourse.bass_utils import run_bass_kernel_spmd

BF16 = mybir.dt.bfloat16
FP8 = mybir.dt.float8e4
NPFP8 = ml_dtypes.float8_e4m3
SCALE = 256.0
FOLD = 16.0                 # u slot scale (matches strip*slot psum scale)
VSCALE = SCALE * FOLD       # 4096: final psum = VSCALE * out
F32 = mybir.dt.float32
NPBF16 = ml_dtypes.bfloat16

N = 2048          # graph nodes
F = 128           # input_size (64 input + 64 hidden)
B = 64            # global batch
NCORES = 8
BS = B // NCORES  # 8 batches per core
O = 64            # output features
NT = N // 128     # 16 node tiles
M5 = 5            # diffusion matrices


def _legalize_waits(nc, max_waits=1):
    """Walrus in this env encodes at most one sync-wait per instruction.

    Tile's sem assignment can emit 2-3 waits on one instruction; hoist the
    excess onto standalone EventSemaphore carriers (same engine, inserted
    just before), which the sequencer executes in order — semantics are
    identical, encoding is legal."""
    f = nc.m.functions[0]
    for blk in f.blocks:
        new_insts = []
        changed = False
        for inst in blk.instructions:
            si = inst.sync_info
            waits = list(si.on_wait) if si is not None else []
            if len(waits) > max_waits:
                for i, w in enumerate(waits[:-max_waits]):
                    ev = mybir.InstEventSemaphore(
                        name=f"{inst.name}-wsplit{i}",
                        engine=inst.engine,
                        ins=[],
                        outs=[],
                        sync_info=mybir.SyncInfo(on_wait=[w], on_update=[]),
                    )
                    new_insts.append(ev)
                inst.sync_info = mybir.SyncInfo(
                    on_wait=waits[-max_waits:], on_update=list(si.on_update)
                )
                changed = True
            new_insts.append(inst)
        if changed:
            blk.instructions = new_insts
    return nc


def build_bass(n=N, bs=BS, o=O, legalize=True, repeat=1):
    """Build the per-core SPMD Bass program."""
    nt = n // 128
    nc = bass.Bass()
    # fp8 x0 (phase1 + hop folds) loads first; bf16 x0 (final v0 folds) last.
    x0f = nc.dram_tensor("x0f", [F, bs * n], FP8, kind="ExternalInput")
    x0b = nc.dram_tensor("x0b", [F, bs * n], BF16, kind="ExternalInput")
    wp8 = nc.dram_tensor("wp8", [F, 2 * o], FP8, kind="ExternalInput")
    # wbf slices (64 cols each): [16W1 | 16W3 | 4096*Wv0]
    wbf = nc.dram_tensor("wbf", [F, 3 * o], BF16, kind="ExternalInput")
    s0t = nc.dram_tensor("s0t", [128, nt * n], FP8, kind="ExternalInput")
    s1t = nc.dram_tensor("s1t", [128, nt * n], FP8, kind="ExternalInput")
    out = nc.dram_tensor("out", [n, bs * o], BF16, kind="ExternalOutput")

    obs = bs * o        # 512: width of diffusion operands
    with TileContext(nc) as tc:
        with (
            tc.tile_pool(name="persist", bufs=1) as persist,
            tc.tile_pool(name="outp", bufs=4) as outp,
            tc.tile_pool(name="psum", bufs=4, space="PSUM") as psum,
        ):
            w8_sb = persist.tile([F, 2 * o], FP8, name="w8_sb")
            wb_sb = persist.tile([F, 3 * o], BF16, name="wb_sb")
            nc.sync.dma_start(out=w8_sb[:, :], in_=wp8[:, :])
            # x0 images are t-major on host: free index = t*bs*128 + b*128 + j,
            # so each node-tile's stationary slices arrive in one chunk DMA.
            # Chunky DMAs: SP sequencer spends 565ns configuring each DMA, so
            # few-but-large transfers keep config time off the critical path
            # while chunk granularity still lets consumers start early.  The
            # first chunk is a single node-tile so phase 1 starts ASAP; wbf
            # (first needed by the hop folds ~15us in) loads after x0f.
            x0f_sb = persist.tile([F, bs * n], FP8, name="x0f_sb")
            ck = 2 * bs * 128
            nc.sync.dma_start(out=x0f_sb[:, :ck // 2], in_=x0f[:, :ck // 2])
            nc.sync.dma_start(out=x0f_sb[:, ck // 2:ck], in_=x0f[:, ck // 2:ck])
            for c in range(1, nt // 2):
                nc.sync.dma_start(out=x0f_sb[:, c * ck:(c + 1) * ck],
                                  in_=x0f[:, c * ck:(c + 1) * ck])
            nc.sync.dma_start(out=wb_sb[:, :], in_=wbf[:, :])
            # Resident strip sets: loaded once, reused by hop and final phases.
            S0sb = persist.tile([128, nt * n], FP8, name="s0sb")
            S1sb = persist.tile([128, nt * n], FP8, name="s1sb")
            S0 = [S0sb[:, t * n:(t + 1) * n] for t in range(nt)]
            S1 = [S1sb[:, t * n:(t + 1) * n] for t in range(nt)]
            sk = 4 * n
            for c in range(nt // 4):
                nc.sync.dma_start(out=S0sb[:, c * sk:(c + 1) * sk],
                                  in_=s0t[:, c * sk:(c + 1) * sk])
            for c in range(nt // 4):
                nc.sync.dma_start(out=S1sb[:, c * sk:(c + 1) * sk],
                                  in_=s1t[:, c * sk:(c + 1) * sk])
            x0b_sb = persist.tile([F, bs * n], BF16, name="x0b_sb")
            bk = 4 * bs * 128
            for c in range(nt // 4):
                nc.sync.dma_start(out=x0b_sb[:, c * bk:(c + 1) * bk],
                                  in_=x0b[:, c * bk:(c + 1) * bk])
            # U[tp]: [128, 4*2*obs] fp8, slots (mi-1) in {0:c0, 1:2u2/16,
            # 2:c1, 3:2u4/16}, each slot = [kt2, b*o].
            U = [
                persist.tile([128, 4 * 2 * obs], FP8, name=f"u{tp}", tag=f"u{tp}")
                for tp in range(nt // 2)
            ]

            def upair(tp, mi):
                """[128, 2, obs] DoubleRow moving view: k-tile pair of slot mi."""
                return U[tp].rearrange("p (mi4 kt2 c) -> p mi4 kt2 c", mi4=4, kt2=2)[
                    :, mi - 1, :, :
                ]

            def uslot_w(t, mi):
                """[128, obs] contiguous write view of slot mi for node-tile t."""
                base = (mi - 1) * 2 * obs + (t % 2) * obs
                return U[t // 2][:, base:base + obs]

            def xfblk(t, b):
                """fp8 stationary x0 slice for (node-tile t, batch b)."""
                return x0f_sb[:, (t * bs + b) * 128:(t * bs + b + 1) * 128]

            def xbblk(t, b):
                """bf16 stationary x0 slice for (node-tile t, batch b)."""
                return x0b_sb[:, (t * bs + b) * 128:(t * bs + b + 1) * 128]

            # PSUM->SBUF drains rotate across DVE and Activation (gpsimd/Pool
            # cannot read PSUM).
            eng_state = [0]

            def drain(out_ap, in_ap, scale=None):
                e = eng_state[0] % 2
                eng_state[0] += 1
                if scale is None:
                    if e == 0:
                        nc.vector.tensor_copy(out=out_ap, in_=in_ap)
                    else:
                        nc.scalar.copy(out=out_ap, in_=in_ap)
                else:
                    if e == 0:
                        nc.vector.tensor_scalar_mul(out_ap, in_ap, scale)
                    else:
                        nc.scalar.activation(
                            out_ap, in_ap,
                            mybir.ActivationFunctionType.Copy, scale=scale)

            # ---- Phase 1: psum[n, b*(slot,o)] = x0f.T @ [2W2 | 2W4] per
            # node-tile (one 2-bank psum tile, matmuls bank-aligned); two
            # strided 512-col 1/16-scaled drains into the fp8 slots.
            def phase1(t):
                ps = psum.tile([128, 2 * obs], F32, name="ps_p", tag="ps")
                for b in range(bs):
                    nc.tensor.matmul(
                        ps[:, b * 128:(b + 1) * 128],
                        lhsT=xfblk(t, b),
                        rhs=w8_sb[:, :], start=True, stop=True)
                pv = ps.rearrange("p (b8 s2 oo) -> p b8 s2 oo", b8=bs, s2=2)
                for s in range(2):
                    drain(uslot_w(t, 2 * s + 2), pv[:, :, s, :],
                          scale=1.0 / FOLD)

            # ---- Hops: c_dst = u_w + s @ (2 u_src)  [psum at 16x scale]
            #   8 fp8 DoubleRow strip matmuls + 8 fold matmuls (fp8 x0 lhsT,
            #   bf16 16*W rhs) accumulate in one psum group; one fp8 copy out.
            def hop(S, src, wi, dst):
                for t in range(nt):
                    ps = psum.tile([128, obs], F32, name="ps_acc", tag="ps")
                    for ktp in range(nt // 2):
                        nc.tensor.matmul(
                            ps[:, :],
                            lhsT=S[t][:, ktp * 256:(ktp + 1) * 256].rearrange(
                                "p (kt2 j) -> p kt2 j", kt2=2),
                            rhs=upair(ktp, src),
                            start=(ktp == 0),
                            stop=False,
                            perf_mode=mybir.MatmulPerfMode.DoubleRow,
                        )
                    for b in range(bs):
                        nc.tensor.matmul(
                            ps[:, b * o:(b + 1) * o], lhsT=xfblk(t, b),
                            rhs=wb_sb[:, wi * o:(wi + 1) * o],
                            start=False, stop=(b == bs - 1))
                    drain(uslot_w(t, dst), ps[:, :])

            # ---- Final: psum = 4096*(s0@c0 + s1@c1 + v0); scaled bf16 drain.
            def final():
                for t in range(nt):
                    ps = psum.tile([128, obs], F32, name="ps_acc", tag="ps")
                    for g, (S, sl) in enumerate([(S0, 1), (S1, 3)]):
                        for ktp in range(nt // 2):
                            nc.tensor.matmul(
                                ps[:, :],
                                lhsT=S[t][:, ktp * 256:(ktp + 1) * 256].rearrange(
                                    "p (kt2 j) -> p kt2 j", kt2=2),
                                rhs=upair(ktp, sl),
                                start=(g == 0 and ktp == 0),
                                stop=False,
                                perf_mode=mybir.MatmulPerfMode.DoubleRow,
                            )
                    for b in range(bs):
                        nc.tensor.matmul(
                            ps[:, b * o:(b + 1) * o], lhsT=xbblk(t, b),
                            rhs=wb_sb[:, 2 * o:3 * o],
                            start=False, stop=(b == bs - 1))
                    ot = outp.tile([128, obs], BF16, name="ot", tag="ot")
                    drain(ot[:, :], ps[:, :], scale=1.0 / VSCALE)
                    nc.sync.dma_start(
                        out=out[t * 128:(t + 1) * 128, :], in_=ot[:, :])

            # repeat>1 re-runs the whole idempotent pipeline (each round
            # rebuilds U from x0) — used only to measure per-round device
            # time via wall-clock differencing.
            for _rep in range(repeat):
                for t in range(nt):
                    phase1(t)
                hop(S0, 2, 0, 1)   # c0 = u1 + s0 @ (2 u2)
                hop(S1, 4, 1, 3)   # c1 = u3 + s1 @ (2 u4)
                final()
    return _legalize_waits(nc) if legalize else nc


_NC_CACHE = {}


def _get_nc():
    if "nc" not in _NC_CACHE:
        _NC_CACHE["nc"] = build_bass()
    return _NC_CACHE["nc"]


def make_inputs(support0, support1, inputs, state, weight):
    """Host-side layout prep -> per-core in_maps (shared replicated arrays)."""
    xs = np.concatenate(
        [
            np.asarray(inputs, np.float32).reshape(B, N, F // 2),
            np.asarray(state, np.float32).reshape(B, N, F // 2),
        ],
        axis=2,
    )  # [B, N, F]

    w = np.asarray(weight, np.float32).reshape(F, M5, O)
    wv0 = w[:, 0] - w[:, 2] - w[:, 4]
    wp8 = np.concatenate([2.0 * w[:, 2], 2.0 * w[:, 4]], axis=1).astype(NPFP8)
    wbf = np.concatenate(
        [FOLD * w[:, 1], FOLD * w[:, 3], VSCALE * wv0], axis=1
    ).astype(NPBF16)  # [128, 192]

    def strip_img(s):
        # fp8 DoubleRow pair layout: [p, t*2048 + ktp*256 + kt2*128 + j]
        #   = fp8(SCALE * s[t*128+j, (ktp*2+kt2)*128 + p])
        r = (SCALE * np.asarray(s, np.float32)).astype(NPFP8)
        r = r.reshape(NT, 128, NT, 128).transpose(3, 0, 2, 1)  # [p, t, kt, j]
        return np.ascontiguousarray(r.reshape(128, NT * N))

    s0i, s1i = strip_img(support0), strip_img(support1)

    in_maps = []
    for c in range(NCORES):
        shard = xs[c * BS:(c + 1) * BS]                # [8b, N, F]
        # t-major SBUF image: x0[f, t*BS*128 + b*128 + j] = shard[b, t*128+j, f]
        x0img = np.ascontiguousarray(
            shard.reshape(BS, NT, 128, F).transpose(3, 1, 0, 2).reshape(F, BS * N)
        )
        in_maps.append({
            "x0f": x0img.astype(NPFP8), "x0b": x0img.astype(NPBF16),
            "wp8": wp8, "wbf": wbf, "s0t": s0i, "s1t": s1i,
        })
    return in_maps


def postprocess(results, biases):
    full = np.empty((B, N, O), np.float32)
    for c, r in enumerate(results):
        full[c * BS:(c + 1) * BS] = (
            r["out"].astype(np.float32).reshape(N, BS, O).transpose(1, 0, 2)
        )
    full += np.asarray(biases, np.float32)[None, None, :]
    return full.reshape(B, N * O)


def kernel(support0, support1, inputs, state, weight, biases, output_size=None,
           **run_kwargs):
    nc = _get_nc()
    in_maps = make_inputs(support0, support1, inputs, state, weight)
    res = run_bass_kernel_spmd(nc, in_maps, core_ids=list(range(NCORES)),
                               **run_kwargs)
    out = postprocess(res.results, biases)
    if run_kwargs.get("trace"):
        return out, res
    return out


# revision 24
# speedup vs baseline: 702.2534x; 702.2534x over previous
"""DiffusionGraphConv Trainium2 kernel (8-core SPMD, data-parallel over batch).

Math refactoring (halves the big-matmul FLOPs vs the reference order):
  reference: out[b,n,o] = sum_{f,m} mats_m[n,f,b] * W[f*5+m, o]
  with mats = [x0, s0 x0, 2 s0^2 x0 - x0, s1 x0, 2 s1^2 x0 - x0].
  Projection (width F=128 -> O=64) commutes with the node-space diffusion, so
  with u_m = x0 @ W_m:
    out = (u0 - u2 - u4) + s0 (u1 + 2 s0 u2) + s1 (u3 + 2 s1 u4)
        = v0 + s0 @ c0 + s1 @ c1,  c0 = u1 + s0 @ (2 u2), c1 = u3 + s1 @ (2 u4)

Execution structure (v3 — PE-engine-saturated):
  * u1 / u3 / v0 are never materialized: they are folded into the diffusion
    PSUM groups as 8 extra 64-col matmuls (lhsT = x0 tile, rhs = a pre-scaled
    W slice) appended after the fp8 DoubleRow strip matmuls.  Each hop ends
    in ONE psum->SBUF copy.
  * Dual x0 image: phase 1 and the hop folds read an fp8 x0 (2MB, loaded
    first so the startup window is DMA-short); only the final phase's v0
    folds — the accuracy-dominant direct term — use the bf16 x0, which loads
    after the strips and arrives long before it is needed.  (u2/u4 are
    diffused twice and u1/u3 once, so their fp8 x0 noise averages out.)
  * Both support strip sets stay SBUF-resident (loaded once, 64KB/partition):
    strip HBM traffic is 8MB/core and the final phase has no DMA dependence.
  * PSUM->SBUF drains rotate across DVE / Activation.
  * Output is written bf16 (host accumulates in f32 and adds biases).

Per-core PE work: 512 fp8-DoubleRow matmuls (the 54.6us streaming floor)
+ 512 small matmuls; PE engine busy ~72us is the modeled bottleneck.

Scales (fp8-safety): strips = fp8(256*s); phase1 psum = 2*u{2,4}, drained
with scale 1/16 -> slots fp8(2u/16); hop psum = 16*c{0,1} stored fp8 (fold
rhs = 16*W{1,3} bf16); final psum = 256*16*(s@c) + 4096*v0 = 4096*out,
drained with a 1/4096 scaled copy.

Env quirks handled here: walrus accepts <=1 sync-wait per instruction
(_legalize_waits hoists extras onto EventSemaphore carriers; simulators need
legalize=False); repeat=N re-runs the idempotent pipeline for wall-clock
differencing since this axon terminal has no NTFF profiling.
"""

import sys

if "/opt/trn_rl_repo" not in sys.path:
    sys.path.insert(0, "/opt/trn_rl_repo")

import numpy as np
import ml_dtypes

import concourse.bass as bass
import concourse.mybir as mybir
from concourse.tile import TileContext
from concourse.bass_utils import run_bass_kernel_spmd

BF16 = mybir.dt.bfloat16
FP8 = mybir.dt.float8e4
NPFP8 = ml_dtypes.float8_e4m3
SCALE = 256.0
FOLD = 16.0                 # u slot scale (matches strip*slot psum scale)
VSCALE = SCALE * FOLD       # 4096: final psum = VSCALE * out
F32 = mybir.dt.float32
NPBF16 = ml_dtypes.bfloat16

N = 2048          # graph nodes
F = 128           # input_size (64 input + 64 hidden)
B = 64            # global batch
NCORES = 8
BS = B // NCORES  # 8 batches per core
O = 64            # output features
NT = N // 128     # 16 node tiles
M5 = 5            # diffusion matrices


def _legalize_waits(nc, max_waits=1):
    """Walrus in this env encodes at most one sync-wait per instruction.

    Tile's sem assignment can emit 2-3 waits on one instruction; hoist the
    excess onto standalone EventSemaphore carriers (same engine, inserted
    just before), which the sequencer executes in order — semantics are
    identical, encoding is legal."""
    f = nc.m.functions[0]
    for blk in f.blocks:
        new_insts = []
        changed = False
        for inst in blk.instructions:
            si = inst.sync_info
            waits = list(si.on_wait) if si is not None else []
            if len(waits) > max_waits:
                for i, w in enumerate(waits[:-max_waits]):
                    ev = mybir.InstEventSemaphore(
                        name=f"{inst.name}-wsplit{i}",
                        engine=inst.engine,
                        ins=[],
                        outs=[],
                        sync_info=mybir.SyncInfo(on_wait=[w], on_update=[]),
                    )
                    new_insts.append(ev)
                inst.sync_info = mybir.SyncInfo(
                    on_wait=waits[-max_waits:], on_update=list(si.on_update)
                )
                changed = True
            new_insts.append(inst)
        if changed:
            blk.instructions = new_insts
    return nc


def build_bass(n=N, bs=BS, o=O, legalize=True, repeat=1):
    """Build the per-core SPMD Bass program."""
    nt = n // 128
    nc = bass.Bass()
    # fp8 x0 (phase1 + hop folds) loads first; bf16 x0 (final v0 folds) last.
    x0f = nc.dram_tensor("x0f", [F, bs * n], FP8, kind="ExternalInput")
    x0b = nc.dram_tensor("x0b", [F, bs * n], BF16, kind="ExternalInput")
    wp8 = nc.dram_tensor("wp8", [F, 2 * o], FP8, kind="ExternalInput")
    # wbf slices (64 cols each): [16W1 | 16W3 | 4096*Wv0]
    wbf = nc.dram_tensor("wbf", [F, 3 * o], BF16, kind="ExternalInput")
    s0t = nc.dram_tensor("s0t", [128, nt * n], FP8, kind="ExternalInput")
    s1t = nc.dram_tensor("s1t", [128, nt * n], FP8, kind="ExternalInput")
    out = nc.dram_tensor("out", [n, bs * o], BF16, kind="ExternalOutput")

    obs = bs * o        # 512: width of diffusion operands
    with TileContext(nc) as tc:
        with (
            tc.tile_pool(name="persist", bufs=1) as persist,
            tc.tile_pool(name="outp", bufs=4) as outp,
            tc.tile_pool(name="psum", bufs=4, space="PSUM") as psum,
        ):
            w8_sb = persist.tile([F, 2 * o], FP8, name="w8_sb")
            wb_sb = persist.tile([F, 3 * o], BF16, name="wb_sb")
            warm_sb = persist.tile([128, 128], FP8, name="warm_sb")
            nc.gpsimd.memset(warm_sb[:, :], 0.0)
            nc.sync.dma_start(out=w8_sb[:, :], in_=wp8[:, :])
            # x0 images are t-major on host: free index = t*bs*128 + b*128 + j,
            # so each node-tile's stationary slices arrive in one chunk DMA.
            # Chunky DMAs: SP sequencer spends 565ns configuring each DMA, so
            # few-but-large transfers keep config time off the critical path
            # while chunk granularity still lets consumers start early.  The
            # first chunk is a single node-tile so phase 1 starts ASAP; wbf
            # (first needed by the hop folds ~15us in) loads after x0f.
            x0f_sb = persist.tile([F, bs * n], FP8, name="x0f_sb")
            ck = 2 * bs * 128
            nc.sync.dma_start(out=x0f_sb[:, :ck // 2], in_=x0f[:, :ck // 2])
            nc.sync.dma_start(out=x0f_sb[:, ck // 2:ck], in_=x0f[:, ck // 2:ck])
            for c in range(1, nt // 2):
                nc.sync.dma_start(out=x0f_sb[:, c * ck:(c + 1) * ck],
                                  in_=x0f[:, c * ck:(c + 1) * ck])
            nc.sync.dma_start(out=wb_sb[:, :], in_=wbf[:, :])
            # Resident strip sets: loaded once, reused by hop and final phases.
            S0sb = persist.tile([128, nt * n], FP8, name="s0sb")
            S1sb = persist.tile([128, nt * n], FP8, name="s1sb")
            S0 = [S0sb[:, t * n:(t + 1) * n] for t in range(nt)]
            S1 = [S1sb[:, t * n:(t + 1) * n] for t in range(nt)]
            sk = 4 * n
            for c in range(nt // 4):
                nc.sync.dma_start(out=S0sb[:, c * sk:(c + 1) * sk],
                                  in_=s0t[:, c * sk:(c + 1) * sk])
            for c in range(nt // 4):
                nc.sync.dma_start(out=S1sb[:, c * sk:(c + 1) * sk],
                                  in_=s1t[:, c * sk:(c + 1) * sk])
            x0b_sb = persist.tile([F, bs * n], BF16, name="x0b_sb")
            bk = 4 * bs * 128
            for c in range(nt // 4):
                nc.sync.dma_start(out=x0b_sb[:, c * bk:(c + 1) * bk],
                                  in_=x0b[:, c * bk:(c + 1) * bk])
            # U[tp]: [128, 4*2*obs] fp8, slots (mi-1) in {0:c0, 1:2u2/16,
            # 2:c1, 3:2u4/16}, each slot = [kt2, b*o].
            U = [
                persist.tile([128, 4 * 2 * obs], FP8, name=f"u{tp}", tag=f"u{tp}")
                for tp in range(nt // 2)
            ]

            def upair(tp, mi):
                """[128, 2, obs] DoubleRow moving view: k-tile pair of slot mi."""
                return U[tp].rearrange("p (mi4 kt2 c) -> p mi4 kt2 c", mi4=4, kt2=2)[
                    :, mi - 1, :, :
                ]

            def uslot_w(t, mi):
                """[128, obs] contiguous write view of slot mi for node-tile t."""
                base = (mi - 1) * 2 * obs + (t % 2) * obs
                return U[t // 2][:, base:base + obs]

            def xfblk(t, b):
                """fp8 stationary x0 slice for (node-tile t, batch b)."""
                return x0f_sb[:, (t * bs + b) * 128:(t * bs + b + 1) * 128]

            def xbblk(t, b):
                """bf16 stationary x0 slice for (node-tile t, batch b)."""
                return x0b_sb[:, (t * bs + b) * 128:(t * bs + b + 1) * 128]

            # PSUM->SBUF drains rotate across DVE and Activation (gpsimd/Pool
            # cannot read PSUM).
            eng_state = [0]

            def drain(out_ap, in_ap, scale=None):
                e = eng_state[0] % 2
                eng_state[0] += 1
                if scale is None:
                    if e == 0:
                        nc.vector.tensor_copy(out=out_ap, in_=in_ap)
                    else:
                        nc.scalar.copy(out=out_ap, in_=in_ap)
                else:
                    if e == 0:
                        nc.vector.tensor_scalar_mul(out_ap, in_ap, scale)
                    else:
                        nc.scalar.activation(
                            out_ap, in_ap,
                            mybir.ActivationFunctionType.Copy, scale=scale)

            # PE p-state warmup: the tensor engine ramps 0.65->1.2->2.4GHz
            # over ~3us of continuous execution.  Dummy matmuls on the tiny
            # already-loaded w8 tile fill the boot DMA window so real work
            # starts at full clock.  Their psum tile is never read.
            def warmup(k):
                ps = psum.tile([128, 2 * obs], F32, name="ps_w", tag="ps")
                for _ in range(k):
                    nc.tensor.matmul(ps[:, 0:128], lhsT=warm_sb[:, :],
                                     rhs=warm_sb[:, :], start=True, stop=True)

            # ---- Phase 1: psum[n, b*(slot,o)] = x0f.T @ [2W2 | 2W4] per
            # node-tile (one 2-bank psum tile, matmuls bank-aligned); two
            # strided 512-col 1/16-scaled drains into the fp8 slots.
            def phase1(t):
                ps = psum.tile([128, 2 * obs], F32, name="ps_p", tag="ps")
                for b in range(bs):
                    nc.tensor.matmul(
                        ps[:, b * 128:(b + 1) * 128],
                        lhsT=xfblk(t, b),
                        rhs=w8_sb[:, :], start=True, stop=True)
                pv = ps.rearrange("p (b8 s2 oo) -> p b8 s2 oo", b8=bs, s2=2)
                for s in range(2):
                    drain(uslot_w(t, 2 * s + 2), pv[:, :, s, :],
                          scale=1.0 / FOLD)

            # ---- Hops: c_dst = u_w + s @ (2 u_src)  [psum at 16x scale]
            #   8 fp8 DoubleRow strip matmuls + 8 fold matmuls (fp8 x0 lhsT,
            #   bf16 16*W rhs) accumulate in one psum group; one fp8 copy out.
            def hop(S, src, wi, dst):
                for t in range(nt):
                    ps = psum.tile([128, obs], F32, name="ps_acc", tag="ps")
                    for ktp in range(nt // 2):
                        nc.tensor.matmul(
                            ps[:, :],
                            lhsT=S[t][:, ktp * 256:(ktp + 1) * 256].rearrange(
                                "p (kt2 j) -> p kt2 j", kt2=2),
                            rhs=upair(ktp, src),
                            start=(ktp == 0),
                            stop=False,
                            perf_mode=mybir.MatmulPerfMode.DoubleRow,
                        )
                    for b in range(bs):
                        nc.tensor.matmul(
                            ps[:, b * o:(b + 1) * o], lhsT=xfblk(t, b),
                            rhs=wb_sb[:, wi * o:(wi + 1) * o],
                            start=False, stop=(b == bs - 1))
                    drain(uslot_w(t, dst), ps[:, :])

            # ---- Final: psum = 4096*(s0@c0 + s1@c1 + v0); scaled bf16 drain.
            def final():
                for t in range(nt):
                    ps = psum.tile([128, obs], F32, name="ps_acc", tag="ps")
                    for g, (S, sl) in enumerate([(S0, 1), (S1, 3)]):
                        for ktp in range(nt // 2):
                            nc.tensor.matmul(
                                ps[:, :],
                                lhsT=S[t][:, ktp * 256:(ktp + 1) * 256].rearrange(
                                    "p (kt2 j) -> p kt2 j", kt2=2),
                                rhs=upair(ktp, sl),
                                start=(g == 0 and ktp == 0),
                                stop=False,
                                perf_mode=mybir.MatmulPerfMode.DoubleRow,
                            )
                    for b in range(bs):
                        nc.tensor.matmul(
                            ps[:, b * o:(b + 1) * o], lhsT=xbblk(t, b),
                            rhs=wb_sb[:, 2 * o:3 * o],
                            start=False, stop=(b == bs - 1))
                    ot = outp.tile([128, obs], BF16, name="ot", tag="ot")
                    drain(ot[:, :], ps[:, :], scale=1.0 / VSCALE)
                    nc.sync.dma_start(
                        out=out[t * 128:(t + 1) * 128, :], in_=ot[:, :])

            # repeat>1 re-runs the whole idempotent pipeline (each round
            # rebuilds U from x0) — used only to measure per-round device
            # time via wall-clock differencing.
            warmup(22)
            for _rep in range(repeat):
                for t in range(nt):
                    phase1(t)
                hop(S0, 2, 0, 1)   # c0 = u1 + s0 @ (2 u2)
                hop(S1, 4, 1, 3)   # c1 = u3 + s1 @ (2 u4)
                final()
    return _legalize_waits(nc) if legalize else nc


_NC_CACHE = {}


def _get_nc():
    if "nc" not in _NC_CACHE:
        _NC_CACHE["nc"] = build_bass()
    return _NC_CACHE["nc"]


def make_inputs(support0, support1, inputs, state, weight):
    """Host-side layout prep -> per-core in_maps (shared replicated arrays)."""
    xs = np.concatenate(
        [
            np.asarray(inputs, np.float32).reshape(B, N, F // 2),
            np.asarray(state, np.float32).reshape(B, N, F // 2),
        ],
        axis=2,
    )  # [B, N, F]

    w = np.asarray(weight, np.float32).reshape(F, M5, O)
    wv0 = w[:, 0] - w[:, 2] - w[:, 4]
    wp8 = np.concatenate([2.0 * w[:, 2], 2.0 * w[:, 4]], axis=1).astype(NPFP8)
    wbf = np.concatenate(
        [FOLD * w[:, 1], FOLD * w[:, 3], VSCALE * wv0], axis=1
    ).astype(NPBF16)  # [128, 192]

    def strip_img(s):
        # fp8 DoubleRow pair layout: [p, t*2048 + ktp*256 + kt2*128 + j]
        #   = fp8(SCALE * s[t*128+j, (ktp*2+kt2)*128 + p])
        r = (SCALE * np.asarray(s, np.float32)).astype(NPFP8)
        r = r.reshape(NT, 128, NT, 128).transpose(3, 0, 2, 1)  # [p, t, kt, j]
        return np.ascontiguousarray(r.reshape(128, NT * N))

    s0i, s1i = strip_img(support0), strip_img(support1)

    in_maps = []
    for c in range(NCORES):
        shard = xs[c * BS:(c + 1) * BS]                # [8b, N, F]
        # t-major SBUF image: x0[f, t*BS*128 + b*128 + j] = shard[b, t*128+j, f]
        x0img = np.ascontiguousarray(
            shard.reshape(BS, NT, 128, F).transpose(3, 1, 0, 2).reshape(F, BS * N)
        )
        in_maps.append({
            "x0f": x0img.astype(NPFP8), "x0b": x0img.astype(NPBF16),
            "wp8": wp8, "wbf": wbf, "s0t": s0i, "s1t": s1i,
        })
    return in_maps


def postprocess(results, biases):
    full = np.empty((B, N, O), np.float32)
    for c, r in enumerate(results):
        full[c * BS:(c + 1) * BS] = (
            r["out"].astype(np.float32).reshape(N, BS, O).transpose(1, 0, 2)
        )
    full += np.asarray(biases, np.float32)[None, None, :]
    return full.reshape(B, N * O)


def kernel(support0, support1, inputs, state, weight, biases, output_size=None,
           **run_kwargs):
    nc = _get_nc()
    in_maps = make_inputs(support0, support1, inputs, state, weight)
    res = run_bass_kernel_spmd(nc, in_maps, core_ids=list(range(NCORES)),
                               **run_kwargs)
    out = postprocess(res.results, biases)
    if run_kwargs.get("trace"):
        return out, res
    return out


# revision 25
# speedup vs baseline: 731.6356x; 1.0418x over previous
"""DiffusionGraphConv Trainium2 kernel (8-core SPMD, data-parallel over batch).

Math refactoring (halves the big-matmul FLOPs vs the reference order):
  reference: out[b,n,o] = sum_{f,m} mats_m[n,f,b] * W[f*5+m, o]
  with mats = [x0, s0 x0, 2 s0^2 x0 - x0, s1 x0, 2 s1^2 x0 - x0].
  Projection (width F=128 -> O=64) commutes with the node-space diffusion, so
  with u_m = x0 @ W_m:
    out = (u0 - u2 - u4) + s0 (u1 + 2 s0 u2) + s1 (u3 + 2 s1 u4)
        = v0 + s0 @ c0 + s1 @ c1,  c0 = u1 + s0 @ (2 u2), c1 = u3 + s1 @ (2 u4)

Execution structure (v3 — PE-engine-saturated):
  * u1 / u3 / v0 are never materialized: they are folded into the diffusion
    PSUM groups as 8 extra 64-col matmuls (lhsT = x0 tile, rhs = a pre-scaled
    W slice) appended after the fp8 DoubleRow strip matmuls.  Each hop ends
    in ONE psum->SBUF copy.
  * Dual x0 image: phase 1 and the hop folds read an fp8 x0 (2MB, loaded
    first so the startup window is DMA-short); only the final phase's v0
    folds — the accuracy-dominant direct term — use the bf16 x0, which loads
    after the strips and arrives long before it is needed.  (u2/u4 are
    diffused twice and u1/u3 once, so their fp8 x0 noise averages out.)
  * Both support strip sets stay SBUF-resident (loaded once, 64KB/partition):
    strip HBM traffic is 8MB/core and the final phase has no DMA dependence.
  * PSUM->SBUF drains rotate across DVE / Activation.
  * Output is written bf16 (host accumulates in f32 and adds biases).

Per-core PE work: 512 fp8-DoubleRow matmuls (the 54.6us streaming floor)
+ 512 small matmuls; PE engine busy ~72us is the modeled bottleneck.

Scales (fp8-safety): strips = fp8(256*s); phase1 psum = 2*u{2,4}, drained
with scale 1/16 -> slots fp8(2u/16); hop psum = 16*c{0,1} stored fp8 (fold
rhs = 16*W{1,3} bf16); final psum = 256*16*(s@c) + 4096*v0 = 4096*out,
drained with a 1/4096 scaled copy.

Env quirks handled here: walrus accepts <=1 sync-wait per instruction
(_legalize_waits hoists extras onto EventSemaphore carriers; simulators need
legalize=False); repeat=N re-runs the idempotent pipeline for wall-clock
differencing since this axon terminal has no NTFF profiling.
"""

import sys

if "/opt/trn_rl_repo" not in sys.path:
    sys.path.insert(0, "/opt/trn_rl_repo")

import numpy as np
import ml_dtypes

import concourse.bass as bass
import concourse.mybir as mybir
from concourse.tile import TileContext
from concourse.bass_utils import run_bass_kernel_spmd

BF16 = mybir.dt.bfloat16
FP8 = mybir.dt.float8e4
NPFP8 = ml_dtypes.float8_e4m3
SCALE = 256.0
FOLD = 16.0                 # u slot scale (matches strip*slot psum scale)
VSCALE = SCALE * FOLD       # 4096: final psum = VSCALE * out
F32 = mybir.dt.float32
NPBF16 = ml_dtypes.bfloat16

N = 2048          # graph nodes
F = 128           # input_size (64 input + 64 hidden)
B = 64            # global batch
NCORES = 8
BS = B // NCORES  # 8 batches per core
O = 64            # output features
NT = N // 128     # 16 node tiles
M5 = 5            # diffusion matrices


def _legalize_waits(nc, max_waits=1):
    """Walrus in this env encodes at most one sync-wait per instruction.

    Tile's sem assignment can emit 2-3 waits on one instruction; hoist the
    excess onto standalone EventSemaphore carriers (same engine, inserted
    just before), which the sequencer executes in order — semantics are
    identical, encoding is legal."""
    f = nc.m.functions[0]
    for blk in f.blocks:
        new_insts = []
        changed = False
        for inst in blk.instructions:
            si = inst.sync_info
            waits = list(si.on_wait) if si is not None else []
            if len(waits) > max_waits:
                for i, w in enumerate(waits[:-max_waits]):
                    ev = mybir.InstEventSemaphore(
                        name=f"{inst.name}-wsplit{i}",
                        engine=inst.engine,
                        ins=[],
                        outs=[],
                        sync_info=mybir.SyncInfo(on_wait=[w], on_update=[]),
                    )
                    new_insts.append(ev)
                inst.sync_info = mybir.SyncInfo(
                    on_wait=waits[-max_waits:], on_update=list(si.on_update)
                )
                changed = True
            new_insts.append(inst)
        if changed:
            blk.instructions = new_insts
    return nc


def build_bass(n=N, bs=BS, o=O, legalize=True, repeat=1):
    """Build the per-core SPMD Bass program."""
    nt = n // 128
    nc = bass.Bass()
    # fp8 x0 (phase1 + hop folds) loads first; bf16 x0 (final v0 folds) last.
    # x0f is a split-f image on 64 partitions: [p<64, t*2048 + b*256 + r*128
    # + j] = x0[b, t*128+j, r*64+p], so DoubleRow (2 f-rows per partition)
    # applies to the 128-deep projection contractions as well.
    x0f = nc.dram_tensor("x0f", [F // 2, 2 * bs * n], FP8, kind="ExternalInput")
    x0b = nc.dram_tensor("x0b", [F, bs * n], BF16, kind="ExternalInput")
    # wp8 [p<64, r*128 + slot*64 + o] = [2W2 | 2W4][r*64+p, slot*64+o]
    wp8 = nc.dram_tensor("wp8", [F // 2, 4 * o], FP8, kind="ExternalInput")
    # wf8 [p<64, wi*128 + r*64 + o] = 16*W{1,3}[r*64+p, o]
    wf8 = nc.dram_tensor("wf8", [F // 2, 4 * o], FP8, kind="ExternalInput")
    # wbf: [4096*Wv0]
    wbf = nc.dram_tensor("wbf", [F, o], BF16, kind="ExternalInput")
    s0t = nc.dram_tensor("s0t", [128, nt * n], FP8, kind="ExternalInput")
    s1t = nc.dram_tensor("s1t", [128, nt * n], FP8, kind="ExternalInput")
    out = nc.dram_tensor("out", [n, bs * o], BF16, kind="ExternalOutput")

    obs = bs * o        # 512: width of diffusion operands
    with TileContext(nc) as tc:
        with (
            tc.tile_pool(name="persist", bufs=1) as persist,
            tc.tile_pool(name="outp", bufs=4) as outp,
            tc.tile_pool(name="psum", bufs=4, space="PSUM") as psum,
        ):
            w8_sb = persist.tile([F // 2, 4 * o], FP8, name="w8_sb")
            wf_sb = persist.tile([F // 2, 4 * o], FP8, name="wf_sb")
            wb_sb = persist.tile([F, o], BF16, name="wb_sb")
            warm_sb = persist.tile([128, 128], FP8, name="warm_sb")
            nc.gpsimd.memset(warm_sb[:, :], 0.0)
            nc.sync.dma_start(out=w8_sb[:, :], in_=wp8[:, :])
            # x0 images are t-major on host: free index = t*bs*128 + b*128 + j,
            # so each node-tile's stationary slices arrive in one chunk DMA.
            # Chunky DMAs: SP sequencer spends 565ns configuring each DMA, so
            # few-but-large transfers keep config time off the critical path
            # while chunk granularity still lets consumers start early.  The
            # first chunk is a single node-tile so phase 1 starts ASAP; wbf
            # (first needed by the hop folds ~15us in) loads after x0f.
            x0f_sb = persist.tile([F // 2, 2 * bs * n], FP8, name="x0f_sb")
            ck = 2 * 2 * bs * 128
            nc.sync.dma_start(out=x0f_sb[:, :ck // 2], in_=x0f[:, :ck // 2])
            nc.sync.dma_start(out=x0f_sb[:, ck // 2:ck], in_=x0f[:, ck // 2:ck])
            for c in range(1, nt // 2):
                nc.sync.dma_start(out=x0f_sb[:, c * ck:(c + 1) * ck],
                                  in_=x0f[:, c * ck:(c + 1) * ck])
            nc.sync.dma_start(out=wf_sb[:, :], in_=wf8[:, :])
            nc.sync.dma_start(out=wb_sb[:, :], in_=wbf[:, :])
            # Resident strip sets: loaded once, reused by hop and final phases.
            S0sb = persist.tile([128, nt * n], FP8, name="s0sb")
            S1sb = persist.tile([128, nt * n], FP8, name="s1sb")
            S0 = [S0sb[:, t * n:(t + 1) * n] for t in range(nt)]
            S1 = [S1sb[:, t * n:(t + 1) * n] for t in range(nt)]
            sk = 4 * n
            for c in range(nt // 4):
                nc.sync.dma_start(out=S0sb[:, c * sk:(c + 1) * sk],
                                  in_=s0t[:, c * sk:(c + 1) * sk])
            for c in range(nt // 4):
                nc.sync.dma_start(out=S1sb[:, c * sk:(c + 1) * sk],
                                  in_=s1t[:, c * sk:(c + 1) * sk])
            x0b_sb = persist.tile([F, bs * n], BF16, name="x0b_sb")
            bk = 4 * bs * 128
            for c in range(nt // 4):
                nc.sync.dma_start(out=x0b_sb[:, c * bk:(c + 1) * bk],
                                  in_=x0b[:, c * bk:(c + 1) * bk])
            # U[tp]: [128, 4*2*obs] fp8, slots (mi-1) in {0:c0, 1:2u2/16,
            # 2:c1, 3:2u4/16}, each slot = [kt2, b*o].
            U = [
                persist.tile([128, 4 * 2 * obs], FP8, name=f"u{tp}", tag=f"u{tp}")
                for tp in range(nt // 2)
            ]

            def upair(tp, mi):
                """[128, 2, obs] DoubleRow moving view: k-tile pair of slot mi."""
                return U[tp].rearrange("p (mi4 kt2 c) -> p mi4 kt2 c", mi4=4, kt2=2)[
                    :, mi - 1, :, :
                ]

            def uslot_w(t, mi):
                """[128, obs] contiguous write view of slot mi for node-tile t."""
                base = (mi - 1) * 2 * obs + (t % 2) * obs
                return U[t // 2][:, base:base + obs]

            def xfblk(t, b):
                """fp8 split-f DoubleRow lhsT for (node-tile t, batch b):
                [64, 2, 128], (p, r, j) -> x0[b, t*128+j, r*64+p]."""
                base = (t * bs + b) * 256
                return x0f_sb[:, base:base + 256].rearrange(
                    "p (r j) -> p r j", r=2)

            def xbblk(t, b):
                """bf16 stationary x0 slice for (node-tile t, batch b)."""
                return x0b_sb[:, (t * bs + b) * 128:(t * bs + b + 1) * 128]

            # PSUM->SBUF drains rotate across DVE and Activation (gpsimd/Pool
            # cannot read PSUM).
            eng_state = [0]

            def drain(out_ap, in_ap, scale=None):
                e = eng_state[0] % 2
                eng_state[0] += 1
                if scale is None:
                    if e == 0:
                        nc.vector.tensor_copy(out=out_ap, in_=in_ap)
                    else:
                        nc.scalar.copy(out=out_ap, in_=in_ap)
                else:
                    if e == 0:
                        nc.vector.tensor_scalar_mul(out_ap, in_ap, scale)
                    else:
                        nc.scalar.activation(
                            out_ap, in_ap,
                            mybir.ActivationFunctionType.Copy, scale=scale)

            # PE p-state warmup: the tensor engine ramps 0.65->1.2->2.4GHz
            # over ~3us of continuous execution.  Dummy matmuls on the tiny
            # already-loaded w8 tile fill the boot DMA window so real work
            # starts at full clock.  Their psum tile is never read.
            def warmup(k):
                ps = psum.tile([128, 2 * obs], F32, name="ps_w", tag="ps")
                for _ in range(k):
                    nc.tensor.matmul(ps[:, 0:128], lhsT=warm_sb[:, :],
                                     rhs=warm_sb[:, :], start=True, stop=True)

            # ---- Phase 1: psum[n, b*(slot,o)] = x0f.T @ [2W2 | 2W4] per
            # node-tile (one 2-bank psum tile, matmuls bank-aligned); two
            # strided 512-col 1/16-scaled drains into the fp8 slots.
            def phase1(t):
                ps = psum.tile([128, 2 * obs], F32, name="ps_p", tag="ps")
                w8v = w8_sb.rearrange("p (r c) -> p r c", r=2)
                for b in range(bs):
                    nc.tensor.matmul(
                        ps[:, b * 128:(b + 1) * 128],
                        lhsT=xfblk(t, b),
                        rhs=w8v, start=True, stop=True,
                        perf_mode=mybir.MatmulPerfMode.DoubleRow)
                pv = ps.rearrange("p (b8 s2 oo) -> p b8 s2 oo", b8=bs, s2=2)
                for s in range(2):
                    drain(uslot_w(t, 2 * s + 2), pv[:, :, s, :],
                          scale=1.0 / FOLD)

            # ---- Hops: c_dst = u_w + s @ (2 u_src)  [psum at 16x scale]
            #   8 fp8 DoubleRow strip matmuls + 8 fold matmuls (fp8 x0 lhsT,
            #   bf16 16*W rhs) accumulate in one psum group; one fp8 copy out.
            def hop(S, src, wi, dst):
                for t in range(nt):
                    ps = psum.tile([128, obs], F32, name="ps_acc", tag="ps")
                    for ktp in range(nt // 2):
                        nc.tensor.matmul(
                            ps[:, :],
                            lhsT=S[t][:, ktp * 256:(ktp + 1) * 256].rearrange(
                                "p (kt2 j) -> p kt2 j", kt2=2),
                            rhs=upair(ktp, src),
                            start=(ktp == 0),
                            stop=False,
                            perf_mode=mybir.MatmulPerfMode.DoubleRow,
                        )
                    wfv = wf_sb[:, wi * 128:(wi + 1) * 128].rearrange(
                        "p (r c) -> p r c", r=2)
                    for b in range(bs):
                        nc.tensor.matmul(
                            ps[:, b * o:(b + 1) * o], lhsT=xfblk(t, b),
                            rhs=wfv, start=False, stop=(b == bs - 1),
                            perf_mode=mybir.MatmulPerfMode.DoubleRow)
                    drain(uslot_w(t, dst), ps[:, :])

            # ---- Final: psum = 4096*(s0@c0 + s1@c1 + v0); scaled bf16 drain.
            def final():
                for t in range(nt):
                    ps = psum.tile([128, obs], F32, name="ps_acc", tag="ps")
                    for g, (S, sl) in enumerate([(S0, 1), (S1, 3)]):
                        for ktp in range(nt // 2):
                            nc.tensor.matmul(
                                ps[:, :],
                                lhsT=S[t][:, ktp * 256:(ktp + 1) * 256].rearrange(
                                    "p (kt2 j) -> p kt2 j", kt2=2),
                                rhs=upair(ktp, sl),
                                start=(g == 0 and ktp == 0),
                                stop=False,
                                perf_mode=mybir.MatmulPerfMode.DoubleRow,
                            )
                    for b in range(bs):
                        nc.tensor.matmul(
                            ps[:, b * o:(b + 1) * o], lhsT=xbblk(t, b),
                            rhs=wb_sb[:, :],
                            start=False, stop=(b == bs - 1))
                    ot = outp.tile([128, obs], BF16, name="ot", tag="ot")
                    drain(ot[:, :], ps[:, :], scale=1.0 / VSCALE)
                    nc.sync.dma_start(
                        out=out[t * 128:(t + 1) * 128, :], in_=ot[:, :])

            # repeat>1 re-runs the whole idempotent pipeline (each round
            # rebuilds U from x0) — used only to measure per-round device
            # time via wall-clock differencing.
            warmup(22)
            for _rep in range(repeat):
                for t in range(nt):
                    phase1(t)
                hop(S0, 2, 0, 1)   # c0 = u1 + s0 @ (2 u2)
                hop(S1, 4, 1, 3)   # c1 = u3 + s1 @ (2 u4)
                final()
    return _legalize_waits(nc) if legalize else nc


_NC_CACHE = {}


def _get_nc():
    if "nc" not in _NC_CACHE:
        _NC_CACHE["nc"] = build_bass()
    return _NC_CACHE["nc"]


def make_inputs(support0, support1, inputs, state, weight):
    """Host-side layout prep -> per-core in_maps (shared replicated arrays)."""
    xs = np.concatenate(
        [
            np.asarray(inputs, np.float32).reshape(B, N, F // 2),
            np.asarray(state, np.float32).reshape(B, N, F // 2),
        ],
        axis=2,
    )  # [B, N, F]

    w = np.asarray(weight, np.float32).reshape(F, M5, O)
    wv0 = w[:, 0] - w[:, 2] - w[:, 4]

    def fsplit(a):
        # [128, c] -> [64, 2*c]: out[p, r*c + k] = a[r*64+p, k]
        c = a.shape[1]
        return np.ascontiguousarray(
            a.reshape(2, 64, c).transpose(1, 0, 2).reshape(64, 2 * c))

    wp8 = fsplit(
        np.concatenate([2.0 * w[:, 2], 2.0 * w[:, 4]], axis=1)
    ).astype(NPFP8)  # [64, 256], r-major pairs of [2W2 | 2W4]
    wf8 = np.concatenate(
        [fsplit(FOLD * w[:, 1]), fsplit(FOLD * w[:, 3])], axis=1
    ).astype(NPFP8)  # [64, 256] = [16W1 split | 16W3 split]
    wbf = (VSCALE * wv0).astype(NPBF16)  # [128, 64]

    def strip_img(s):
        # fp8 DoubleRow pair layout: [p, t*2048 + ktp*256 + kt2*128 + j]
        #   = fp8(SCALE * s[t*128+j, (ktp*2+kt2)*128 + p])
        r = (SCALE * np.asarray(s, np.float32)).astype(NPFP8)
        r = r.reshape(NT, 128, NT, 128).transpose(3, 0, 2, 1)  # [p, t, kt, j]
        return np.ascontiguousarray(r.reshape(128, NT * N))

    s0i, s1i = strip_img(support0), strip_img(support1)

    in_maps = []
    for c in range(NCORES):
        shard = xs[c * BS:(c + 1) * BS]                # [8b, N, F]
        # t-major SBUF image: x0[f, t*BS*128 + b*128 + j] = shard[b, t*128+j, f]
        x0img = np.ascontiguousarray(
            shard.reshape(BS, NT, 128, F).transpose(3, 1, 0, 2).reshape(F, BS * N)
        )
        # split-f fp8 image: [64, t*2048 + b*256 + r*128 + j]
        x0f2 = np.ascontiguousarray(
            shard.reshape(BS, NT, 128, 2, 64).transpose(4, 1, 0, 3, 2)
            .reshape(64, 2 * BS * N))
        in_maps.append({
            "x0f": x0f2.astype(NPFP8), "x0b": x0img.astype(NPBF16),
            "wp8": wp8, "wf8": wf8, "wbf": wbf, "s0t": s0i, "s1t": s1i,
        })
    return in_maps


def postprocess(results, biases):
    full = np.empty((B, N, O), np.float32)
    for c, r in enumerate(results):
        full[c * BS:(c + 1) * BS] = (
            r["out"].astype(np.float32).reshape(N, BS, O).transpose(1, 0, 2)
        )
    full += np.asarray(biases, np.float32)[None, None, :]
    return full.reshape(B, N * O)


def kernel(support0, support1, inputs, state, weight, biases, output_size=None,
           **run_kwargs):
    nc = _get_nc()
    in_maps = make_inputs(support0, support1, inputs, state, weight)
    res = run_bass_kernel_spmd(nc, in_maps, core_ids=list(range(NCORES)),
                               **run_kwargs)
    out = postprocess(res.results, biases)
    if run_kwargs.get("trace"):
        return out, res
    return out
